# revision 8
# baseline (speedup 1.0000x reference)
"""Trainium2 Bass kernel for batched KNN (B=4, M=8192, N=8192, C=64, k=16).

Sharding: 8 cores = 4 batches x 2 query halves. Each core computes, for its
4096 queries against the full 8192-point support set of its batch, top-8
candidates per 2048-wide selection chunk (32 candidates/row); the host
finishes with an exact top-16 merge over the 32 candidates.

Per-core algorithm:
  r[m,n] = 2*q_m.s_n - |s_n|^2   (maximize r  <=>  minimize squared distance)
  computed on the PE in two fp16 hi/lo-split matmuls accumulated in fp32 PSUM
  (error ~2e-6, at the noise level of fp32 itself):
      MM1 (K=128): [qh;ql]^T . [2sh;2sh]
      MM2 (K=66):  [qh;1;1]^T . [2sl; -sqs_hi; -sqs_lo]
  Hi/lo splits are built in the natural [point, C] layout as interleaved
  [hi|lo] 128-column blocks, so single [128,128] DMA-xbar transposes land
  them directly in the PE-facing layout (no PE/DVE transpose work).
  PSUM chunks ([128,512]) are evicted to SBUF by the scalar engine; the DVE
  extracts top-8 values + positions per 2048-wide selection chunk (hardware
  MAX8/MAX_INDEX_8 over 4 evicted chunks).

The device returns, per query row, the 32 candidate r values and their 32
chunk-local indices.  The host merges to the exact top-16 (argsort over 32),
computes vals = sqrt(relu(|q|^2 - r)), and maps indices globally:
  idx = cand[row, slot] + (slot>>3)*2048.
A 2048-wide chunk only yields its top-8, so a row whose true top-16 has >=9
entries in one chunk is under-covered.  Such rows are exactly detectable
(some chunk contributes all 8 of its candidates to the final 16) and rare
(~3e-4/row); the host recomputes those rows exactly.
"""

import numpy as np

import concourse.bacc as bacc
import concourse.bass as bass
import concourse.mybir as mybir
from concourse import bass_utils
from concourse.masks import make_identity
from concourse.tile import TileContext

F32 = mybir.dt.float32
F16 = mybir.dt.float16
U16 = mybir.dt.uint16
AF = mybir.ActivationFunctionType
SUB = mybir.AluOpType.subtract
MULT = mybir.AluOpType.mult

B, M, N, C = 4, 8192, 8192, 64
NCORES = 8
MC = M // 2  # 4096 query rows per core
K = 16
NEG_INF = -3.0e38

EV = 512   # PSUM bank / eviction chunk
SC = 2048  # selection chunk (DVE top-8 granule)


def build_nc(Mc=MC, Nn=N, debug=False):
    """Build the Bass module for one core (all cores run the same program)."""
    nev = Nn // EV      # eviction chunks per query tile (16)
    nsc = Nn // SC      # selection chunks per query tile (4)
    spc = SC // EV      # evictions per selection chunk (4)
    nt = Mc // 128      # query tiles (32)
    nst = Nn // 128     # support tiles (64)
    ncand = 8 * nsc     # candidate slots per query row (32)

    nc = bacc.Bacc(trn_type="TRN2", target_bir_lowering=False, debug=debug)
    q_d = nc.dram_tensor("query", [Mc, C], F32, kind="ExternalInput")
    s_d = nc.dram_tensor("support", [Nn, C], F32, kind="ExternalInput")
    v1_d = nc.dram_tensor("v1", [Mc, ncand], F32, kind="ExternalOutput")
    i1_d = nc.dram_tensor("i1", [Mc, ncand], U16, kind="ExternalOutput")

    with TileContext(nc) as tc:
        with (
            tc.tile_pool(name="consts", bufs=1) as consts,
            tc.tile_pool(name="psum", bufs=8, space="PSUM") as psum,
            tc.tile_pool(name="rbuf", bufs=2) as rbuf,
            tc.tile_pool(name="small", bufs=3) as small,
        ):
            # PE-facing tensors, writer blocks aligned to matmul read chunks
            QHL = consts.tile([128, Mc], F16)   # [0:64]=qh^T [64:128]=ql^T
            SHT2 = consts.tile([128, Nn], F16)  # 2*s_h^T, replicated twice
            SLQ = consts.tile([66, Nn], F16)    # [0:64]=2*s_l^T [64:66]=-sqs hi/lo
            UH1 = consts.tile([66, Mc], F16)    # [0:64]=qh^T [64:66]=1.0

            with (
                tc.tile_pool(name="stage", bufs=1) as stage,
                tc.tile_pool(name="prep", bufs=3) as prep,
            ):
                ident0 = stage.tile([128, 128], F32)
                make_identity(nc, ident0)
                ident = stage.tile([128, 128], F32)
                nc.vector.tensor_copy(ident, ident0)  # PE reads DVE-written copy
                SQS_T = stage.tile([128, nst], F32)

                # one-shot loads of s and q, tiled [p, t, c]; separate queues
                s_all = stage.tile([128, nst * C], F32)
                nc.sync.dma_start(
                    s_all[:].rearrange("p (t c) -> p t c", c=C),
                    s_d.rearrange("(t p) c -> p t c", p=128),
                )
                q_all = stage.tile([128, nt * C], F32)
                nc.scalar.dma_start(
                    q_all[:].rearrange("p (t c) -> p t c", c=C),
                    q_d.rearrange("(t p) c -> p t c", p=128),
                )

                # hi/lo splits in natural layout, [hi(64) | lo(64)] per tile,
                # so one [128,128] xbar transpose yields the final PE layout
                SHSL = stage.tile([128, 2 * nst * C], F16)
                shsl = SHSL[:].rearrange("p (t w) -> p t w", w=2 * C)
                s_v = s_all[:].rearrange("p (t c) -> p t c", c=C)
                nc.scalar.mul(shsl[:, :, 0:C], s_v, 2.0)
                # 2*s - fp16(2*s) = exact lo residual (2x is exact in fp)
                nc.vector.scalar_tensor_tensor(
                    shsl[:, :, C : 2 * C], s_v, 2.0, shsl[:, :, 0:C],
                    MULT, SUB,
                )
                QHQL = stage.tile([128, 2 * nt * C], F16)
                qhql = QHQL[:].rearrange("p (t w) -> p t w", w=2 * C)
                q_v = q_all[:].rearrange("p (t c) -> p t c", c=C)
                nc.scalar.copy(qhql[:, :, 0:C], q_v)
                nc.vector.scalar_tensor_tensor(
                    qhql[:, :, C : 2 * C], q_v, 1.0, qhql[:, :, 0:C],
                    MULT, SUB,
                )

                # support squared norms: square once, reduce per tile
                scr_s = stage.tile([128, nst * C], F32)
                nc.scalar.activation(scr_s, s_all, AF.Square)
                nc.vector.reduce_sum(
                    SQS_T[:, 0:nst],
                    scr_s[:].rearrange("p (t c) -> p t c", c=C),
                    axis=mybir.AxisListType.X,
                )

                # -|s|^2 in fp16 hi/lo on partitions 64,65 of SLQ -- issued
                # FIRST so its DMAs (which every MM2 waits on) aren't queued
                # behind the transposes
                sqsTc = prep.tile([128, nst], F32, tag="sqsTc")
                nc.vector.tensor_copy(sqsTc, SQS_T[:, 0:nst])
                ps = psum.tile([128, 512], F32, tag="ps")
                nc.tensor.transpose(ps[0:nst, 0:128], sqsTc[:, 0:nst], ident)
                sqsT = prep.tile([nst, 128], F32, tag="sqsT")
                nc.vector.tensor_copy(sqsT, ps[0:nst, 0:128])
                nqh = prep.tile([nst, 128], F16, tag="nqh")
                nq32 = prep.tile([nst, 128], F32, tag="nq32")
                nql = prep.tile([nst, 128], F16, tag="nql")
                nc.vector.tensor_scalar_mul(nqh, sqsT, -1.0)
                nc.vector.tensor_scalar_mul(nq32, sqsT, -1.0)
                nc.vector.tensor_tensor(nql, nq32, nqh, SUB)
                nc.sync.dma_start(SLQ[64:65, :], nqh[0:nst, 0:128])
                nc.sync.dma_start(SLQ[65:66, :], nql[0:nst, 0:128])
                nc.vector.memset(UH1[64:66, :], 1.0)

                def q_transpose(t):
                    eng = nc.sync if t % 2 == 0 else nc.scalar
                    eng.dma_start_transpose(
                        QHL[:, bass.ts(t, 128)], QHQL[:, bass.ts(t, 128)]
                    )
                    eng.dma_start(
                        UH1[0:64, bass.ts(t, 128)], QHL[0:64, bass.ts(t, 128)]
                    )

                # first few query tiles ahead of support so tile 0 can start
                for t in range(4):
                    q_transpose(t)

                # support xbar transposes, chunk-granular so the main loop can
                # start before all of support prep has finished
                for jj in range(nev):
                    ch = bass.ts(jj, EV)
                    for u in range(spc):
                        t = jj * spc + u
                        eng = nc.sync if t % 2 == 0 else nc.scalar
                        eng.dma_start_transpose(
                            SHT2[:, bass.ts(t, 128)], SHSL[:, bass.ts(t, 128)]
                        )
                    # SLQ lo rows from SHT2[64:128] before the hi replicate
                    # overwrites them (Tile orders the WAR hazard)
                    nc.sync.dma_start(SLQ[0:64, ch], SHT2[64:128, ch])
                    nc.scalar.dma_start(SHT2[64:128, ch], SHT2[0:64, ch])

                for t in range(4, nt):
                    q_transpose(t)

            # ---------------- main loop ----------------
            for t in range(nt):
                mcols = bass.ts(t, 128)
                R = rbuf.tile([128, Nn], F32, tag="R")
                V1 = small.tile([128, ncand], F32, tag="V1")
                I1 = small.tile([128, ncand], U16, tag="I1")
                for j in range(nev):
                    ps = psum.tile([128, EV], F32, tag="ps")
                    nc.tensor.matmul(
                        ps, QHL[:, mcols], SHT2[:, bass.ts(j, EV)],
                        start=True, stop=False,
                    )
                    nc.tensor.matmul(
                        ps, UH1[:, mcols], SLQ[:, bass.ts(j, EV)],
                        start=False, stop=True,
                    )
                    nc.scalar.copy(R[:, bass.ts(j, EV)], ps)
                    if j % spc == spc - 1:
                        sj = j // spc
                        nc.vector.max(
                            V1[:, bass.ts(sj, 8)], R[:, bass.ts(sj, SC)]
                        )
                        nc.vector.max_index(
                            I1[:, bass.ts(sj, 8)], V1[:, bass.ts(sj, 8)],
                            R[:, bass.ts(sj, SC)],
                        )
                rows = slice(t * 128, (t + 1) * 128)
                nc.sync.dma_start(v1_d[rows, :], V1)
                nc.sync.dma_start(i1_d[rows, :], I1)
    nc.compile()
    return nc


_BUILT = None


def _get_nc():
    global _BUILT
    if _BUILT is None:
        _BUILT = build_nc()
    return _BUILT


def _fix_suspect_rows(qrows, s_b):
    """Exact numpy KNN for rows where a selection chunk may be under-covered."""
    d2 = (
        (qrows * qrows).sum(1)[:, None]
        + (s_b * s_b).sum(1)[None, :]
        - 2.0 * (qrows @ s_b.T)
    )
    order = np.argsort(d2, axis=1, kind="stable")[:, :K]
    vals = np.sqrt(np.maximum(np.take_along_axis(d2, order, 1), 0.0))
    return vals.astype(np.float32), order.astype(np.int32)


def _assemble(results, query, support):
    nsc = N // SC
    vals = np.empty((B, M, K), np.float32)
    idx = np.empty((B, M, K), np.int32)
    for core in range(NCORES):
        r = results[core]
        b, h = divmod(core, 2)
        v1 = r["v1"]                          # [MC,32] r values (bigger=closer)
        i1 = r["i1"].astype(np.int64)         # [MC,32] chunk-local indices
        # stable argsort on -r: ties keep lower slot (= lower global index)
        order = np.argsort(-v1, axis=1, kind="stable")[:, :K]   # [MC,16]
        rsel = np.take_along_axis(v1, order, 1)
        gi = np.take_along_axis(i1, order, 1) + (order >> 3) * SC
        qrows = query[b, h * MC : (h + 1) * MC]
        sqq = (qrows * qrows).sum(1, dtype=np.float32)
        vals_c = np.sqrt(np.maximum(sqq[:, None] - rsel, 0.0))
        # suspect rows: some chunk contributed all 8 of its candidates
        cnt = (
            (order >> 3)[:, :, None] == np.arange(nsc)[None, None, :]
        ).sum(1)
        bad = (cnt >= 8).any(1)
        if bad.any():
            mrows = np.nonzero(bad)[0]
            fv, fi = _fix_suspect_rows(qrows[mrows], support[b])
            vals_c[mrows] = fv
            gi[mrows] = fi
        vals[b, h * MC : (h + 1) * MC] = vals_c
        idx[b, h * MC : (h + 1) * MC] = gi.astype(np.int32)
    return vals, idx


def kernel(query, support, _trace=False):
    query = np.asarray(query, dtype=np.float32)
    support = np.asarray(support, dtype=np.float32)
    nc = _get_nc()
    in_maps = []
    for core in range(NCORES):
        b, h = divmod(core, 2)
        in_maps.append({
            "query": np.ascontiguousarray(query[b, h * MC : (h + 1) * MC, :]),
            "support": np.ascontiguousarray(support[b]),
        })
    res = bass_utils.run_bass_kernel_spmd(
        nc, in_maps, core_ids=list(range(NCORES)), trace=_trace
    )
    vals, idx = _assemble(res.results, query, support)
    if _trace:
        return (vals, idx), res
    return vals, idx
